# revision 32
# baseline (speedup 1.0000x reference)
"""Trainium2 Bass kernel for nn_DifferentiableParticleFilter (N=8192, 8 cores).

Sharding: the (N,N) soft-resample matrix is sharded by output rows (1024 per
core); the per-particle network + state (N,49) is computed replicated on each
core.  Host pre-transposes each u_gumbel shard so the contraction axis lands
on SBUF partitions.

Big-tensor pipeline per supertile (tau = 0.5 -> exp(g/tau) = 1/ln(u+eps)^2):
    DMA u (fp32) -> scalar Ln in place -> fused custom DVE op
    t = (1/L)^2 (bitnot exponent-flip seed + one tuned NR step + square,
    bf16 out) -> bf16 PE matmul against the weight-folded state.
Phase A (per-particle nets) runs in a 4-way particle-stacked layout
([4*d rows, 2048 cols]) with bf16 matmuls; biases fold into activation bias
columns; particle weights fold into the state via one broadcast TT; the
sigmoid gate is computed as (1+tanh(x/2))/2 so every phase-A transcendental
sits in the silu/tanh table set (4 ACT set switches total).
"""

import numpy as np

import concourse.bass as bass
import concourse.tile as tile
from concourse import bacc, mybir
from concourse.bass_utils import run_bass_kernel_spmd

F32 = mybir.dt.float32
BF16 = mybir.dt.bfloat16
AF = mybir.ActivationFunctionType
ALU = mybir.AluOpType
AX = mybir.AxisListType

K_ACT = 5
EPS = 1.0e-10
LWCLAMP = -30.0
C_LL = float(np.log(2.0) - 0.5 * np.log(2.0 * np.pi))
INV_SQRT2 = float(1.0 / np.sqrt(2.0))

# fused DVE op: out = s0 * ( not_x*(A - B*(x*not_x)) )^2  ~=  s0/x^2
# A,B minimax-tuned for one NR step over u = x*bitcast(~x) in [-4.5,-4].
RSQ_A = -0.47141455934487236
RSQ_B = 0.05546133703759453

B4 = 4                  # particle-stacking factor for phase A
_RSQ_OP = None


def _register_recip_sq():
    """Register the fused (1/x)^2 custom DVE op (idempotent)."""
    global _RSQ_OP
    if _RSQ_OP is not None:
        return _RSQ_OP
    import concourse.dve_ops as dve_ops
    from concourse.dve_ops import DveOp
    from concourse.dve_spec import (AluOp, Bin, C0, C1, C2, Spec, Src0,
                                    lower, _has_src1)
    from concourse.dve_uop import DveOpSpec

    name = "RECIP_SQ_W_ANT"
    for op in dve_ops.OPS:
        if op.name == name:
            _RSQ_OP = op
            return op

    _not = Bin(AluOp.BITWISE_NOT, Src0, Src0)
    _y1 = _not * (C2 - C1 * (Src0 * _not))
    _body = (_y1 * _y1) * C0

    def _ref(in0, in1, c0, c1, c2):
        not_x = (~in0.view(np.int32)).view(np.float32)
        y1 = not_x * (c2 - c1 * (in0 * not_x))
        return (y1 * y1) * c0

    spec = Spec(body=_body, reference=_ref)
    opcode = dve_ops._CUSTOM_DVE_ROW_BASE + len(dve_ops.OPS)
    shas = {}
    for ver in ("v3", "v4"):
        try:
            uops = lower(spec, ver=ver)
            shas[ver] = DveOpSpec(
                name=name, uops=uops, opcode=opcode, rd1_en=_has_src1(spec)
            ).sha(ver)
        except Exception:
            pass
    op = DveOp(name, spec, subdim=False, uops_sha=shas)
    dve_ops.OPS.append(op)
    dve_ops.CUSTOM_DVE_SPECS[name] = spec
    dve_ops._SUB_OPCODE_FOR_NAME[name] = opcode
    _RSQ_OP = op
    return op


# fp32 parameter blob (one DMA): (name, n_partitions, n_cols)
def _param_spec_f32():
    return [
        ("ident", 128, 128),
        ("lhsT_a1", 65, 16), ("lhsT_a2", 16, 1), ("brow_a2", 1, 1),
        ("h_col", 65, 1), ("lhsT_rep5", 5, 128),
        ("log_obs5", 5, 1), ("logR0", 1, 1), ("obs11", 1, 1),
        ("b_x1", 128, 1), ("b_d1h0", 128, 1), ("b_d1h1", 128, 1),
        ("b_d2", 128, 1), ("b_dR", 40, 1), ("b_nlg", 128, 1),
        ("b_g", 128, 1), ("b_c", 128, 1),
        ("rh_p", 128, 64), ("rlow_p", 128, 64), ("eh_p", 128, 64),
        ("el_p", 128, 64), ("lw0_p", 128, 64),
    ]


# bf16 parameter blob: block-diagonal lhsTs for the 4-way stacked layout
def _param_spec_bf16():
    return [
        ("identb", 128, 128),
        ("lhsT_E1r", 60, 128), ("lhsT_E1s", 60, 128),
        ("lhsT_x1", 128, 128),
        ("lhsT_nlgx", 128, 128), ("lhsT_nlgl", 60, 128),
        ("lhsT_d1r0", 128, 128), ("lhsT_d1z0", 128, 128),
        ("lhsT_d1r1", 128, 128), ("lhsT_d1z1", 128, 128),
        ("lhsT_d2a0", 128, 128), ("lhsT_d2a1", 128, 128),
        ("lhsT_d3", 128, 16),
        ("lhsT_gr", 128, 128), ("lhsT_gz", 128, 128),
        ("lhsT_cr", 128, 128), ("lhsT_cz", 128, 128),
    ]


# ---------------------------------------------------------------------------
# device program (SPMD - one program, per-core inputs differ)
# ---------------------------------------------------------------------------

def build_program(n_particles, rows_per_core):
    N = int(n_particles)
    R = int(rows_per_core)
    JT = N // 128                 # 64 j-tiles (contraction tiles of 128)
    CB = N // B4                  # stacked cols per block (2048)
    NCH = CB // 1024              # phase-A 1024-col psum chunks (2)
    G = 4                         # j-tiles per supertile
    SUP = JT // G                 # 16 supertiles
    TW = G * R                    # supertile width (4096)
    NTC = CB // 128               # transpose chunks (16)
    OW = min(128, R)
    OB = R // OW

    rsq = _register_recip_sq()

    nc = bacc.Bacc("TRN2", target_bir_lowering=False, debug=False)

    # Steer the act-table-set chooser: make natural_log_exp_and_others the
    # only set providing Exp and Ln, so exp<->ln alternation never reloads.
    from concourse.hw_specs import get_activation_tables
    _tabs = get_activation_tables(nc.m.arch)
    for _nm, _fns in _tabs.items():
        if _nm != "natural_log_exp_and_others":
            _fns.discard(AF.Exp)
            _fns.discard(AF.Ln)

    def par(name, shape, dt=F32, out=False):
        return nc.declare_dram_parameter(name, list(shape), dt, isOutput=out)

    specf = _param_spec_f32()
    CPf = sum(m for _, _, m in specf)
    specb = _param_spec_bf16()
    CPb = sum(m for _, _, m in specb)
    d_uT = par("uT", (N, R))
    d_z4 = par("z4", (128, CB), BF16)
    d_lg4 = par("lg4", (60, CB), BF16)
    d_pf = par("pf", (128, CPf))
    d_pb = par("pb", (128, CPb), BF16)
    d_y = par("y", (R, 49), out=True)

    with tile.TileContext(nc) as tc:
        _keep = []

        def sm(shape, name, dt=F32):
            t, free = tc.tile(list(shape), dt, name=name)
            _keep.append(free)
            return t

        # ---- persistent tiles -------------------------------------------
        Pf = sm((128, CPf), "Pf")
        nc.sync.dma_start(Pf[:], d_pf[:])
        lg4 = sm((60, CB), "lg4", BF16)
        nc.sync.dma_start(lg4[:], d_lg4[:])
        Pb = sm((128, CPb), "Pb", BF16)
        nc.sync.dma_start(Pb[:], d_pb[:])

        def views(P, spec):
            v, off = {}, 0
            for nm, k, m in spec:
                v[nm] = P[0:k, off:off + m]
                off += m
            return v

        Vf = views(Pf, specf)
        Vb = views(Pb, specb)
        offs = {}
        _o = 0
        for _nm, _k, _m in specf:
            offs[_nm] = _o
            _o += _m
        ident = Vf["ident"]
        identb = Vb["identb"]

        z4 = sm((128, CB), "z4", BF16)
        nc.sync.dma_start(z4[:], d_z4[:])

        state = sm((128, 50 * JT), "state", BF16)
        stg6 = sm((128, 6 * JT), "stg6")
        hl2 = sm((128, 2 * JT), "hl2")
        w_p = sm((128, JT), "w_p")
        eps_col = sm((128, 1), "eps_col")
        nc.vector.memset(eps_col[:], EPS)
        one_col = sm((128, 1), "one_col")
        nc.vector.memset(one_col[:], 1.0)
        ones128 = sm((1, 128), "ones128")
        nc.vector.memset(ones128[:], 1.0)
        L_R4 = sm((128, 8), "L_R4", BF16)
        nc.vector.memset(L_R4[:], 0.0)
        rsr = sm((1, 1), "rsr")
        rsrc_col = sm((128, 1), "rsrc_col")
        obs_col = sm((128, 1), "obs_col")
        e5 = sm((5, 1), "e5")
        scl_col = sm((128, 1), "scl_col")
        ah = sm((16, 1), "ah")
        al_sb = sm((1, 1), "al_sb")
        alpha_col = sm((128, 1), "alpha_col")
        asc = sm((128, 1), "asc")
        warm = sm((1, 1), "warm")
        c001 = sm((128, 1), "c001")
        nc.vector.memset(c001[:], 0.01)
        zero_col = sm((128, 1), "zero_col")
        nc.vector.memset(zero_col[:], 0.0)
        half_col = sm((128, 1), "half_col")
        nc.vector.memset(half_col[:], 0.5)
        nhalf_col = sm((128, 1), "nhalf_col")
        nc.vector.memset(nhalf_col[:], -0.5)
        c015 = sm((128, 1), "c015")
        nc.vector.memset(c015[:], 0.15)
        c400 = sm((128, 1), "c400")
        nc.vector.memset(c400[:], 4.0)
        n30 = sm((128, 1), "n30")
        nc.vector.memset(n30[:], LWCLAMP)
        gsil = sm((1, 1), "gsil")
        gsp = sm((1, 1), "gsp")
        gerf = sm((1, 1), "gerf")
        lwm = sm((128, 1), "lwm")
        lwmax_col = sm((128, 1), "lwmax_col")
        lwrow = sm((1, 128), "lwrow")
        lwm1 = sm((1, 1), "lwm1")

        statemv = state[:, :].rearrange("p (m f) -> p m f", m=JT)
        statebv = state[:, :].rearrange("p (b x) -> p b x", b=B4)
        stg6bv = stg6[:, :].rearrange("p (b x) -> p b x", b=B4)

        # ---- streaming pools (outlive phase A) --------------------------
        with (
            tc.tile_pool(name="ust", bufs=5) as ust,
            tc.tile_pool(name="tst", bufs=7) as tst,
        ):
            uT_r = d_uT.rearrange("(s k p) c -> s p k c", p=128, k=G)
            u_tiles = []
            t_tiles = []
            for s in range(SUP):
                ut = ust.tile([128, TW], F32, tag="u", name=f"u{s}")
                nc.sync.dma_start(ut.rearrange("p (k c) -> p k c", k=G),
                                  uT_r[s])
                u_tiles.append(ut)
                t_tiles.append(
                    tst.tile([128, TW], BF16, tag="t", name=f"t{s}"))

            def ln_sup(s, gate=None):
                if gate is not None:
                    with tc.high_priority(offset=200000):
                        nc.vector.tensor_scalar(
                            u_tiles[s][0:1, 0:1], u_tiles[s][0:1, 0:1],
                            gate[0:1, 0:1], None, ALU.min)
                nc.scalar.activation(u_tiles[s][:], u_tiles[s][:], AF.Ln,
                                     bias=eps_col[:])

            def rsq_sup(s, gate=None):
                nc.vector._custom_dve(rsq, out=t_tiles[s][:],
                                      in0=u_tiles[s][:],
                                      s0=1.0, s1=RSQ_B, imm2=RSQ_A)

            # PE warmer pool: garbage matmuls emitted between phase-A
            # layers keep the tensor-clock ramp from resetting.
            pfil_cm = tc.tile_pool(name="pfil", bufs=1, space="PSUM")
            pfil = pfil_cm.__enter__()
            fil = pfil.tile([64, 512], F32, tag="fil")

            def warm_pe(n=2):
                for _f in range(n):
                    nc.tensor.matmul(fil[:], Pb[0:128, 0:64],
                                     Pb[0:128, 64:576],
                                     start=True, stop=True)

            # =================== phase A =================================
            with (
                tc.tile_pool(name="pha", bufs=1) as pha,
                tc.tile_pool(name="pr2", bufs=1) as pr2,
            ):
                from contextlib import ExitStack
                _psk = ExitStack()
                ppA = _psk.enter_context(
                    tc.tile_pool(name="ppA", bufs=2, space="PSUM"))
                ppB = _psk.enter_context(
                    tc.tile_pool(name="ppB", bufs=1, space="PSUM"))
                ppt = _psk.enter_context(
                    tc.tile_pool(name="ppt", bufs=1, space="PSUM"))
                # --- scalar NL group 1 -----------------------------------
                Esb = pha.tile([60, CB], BF16, tag="Esb")
                nc.scalar.activation(Esb[:], lg4[:], AF.Exp)
                nc.scalar.activation(e5[:], Vf["log_obs5"], AF.Exp)
                nc.scalar.activation(rsr[:], Vf["logR0"], AF.Exp)
                # pre-warm the silu/tanh table while E1/remb run
                nc.scalar.activation(warm[:], Esb[0:1, 0:1], AF.Silu)

                def mms(psum_t, pairs, cs, rows=slice(0, 128)):
                    """psum_t[rows,:1024] += sum_i lhsT_i.T @ rhs_i[:, cs],
                    as 2x512-col matmuls (one PSUM bank each)."""
                    for b5 in range(2):
                        bs = slice(b5 * 512, (b5 + 1) * 512)
                        gs = slice(cs.start + b5 * 512,
                                   cs.start + (b5 + 1) * 512)
                        for i, (lt, rh) in enumerate(pairs):
                            nc.tensor.matmul(psum_t[rows, bs], lt,
                                             rh[:, gs],
                                             start=(i == 0),
                                             stop=(i == len(pairs) - 1))

                def mlp_layer(out_sb, pairs, af, bias_ap, nm):
                    for ch in range(NCH):
                        cs = slice(ch * 1024, (ch + 1) * 1024)
                        ps = ppA.tile([128, 1024], F32, tag="pA",
                                      name=f"{nm}{ch}")
                        mms(ps, pairs, cs)
                        if bias_ap is None:
                            nc.scalar.activation(out_sb[:, cs], ps[:], af)
                        else:
                            nc.scalar.activation(out_sb[:, cs], ps[:], af,
                                                 bias=bias_ap)

                # --- E1: remb = (E @ embed5) / S1 ------------------------
                remb = pha.tile([128, CB], BF16, tag="remb")
                for ch in range(NCH):
                    cs = slice(ch * 1024, (ch + 1) * 1024)
                    p_r = ppA.tile([128, 1024], F32, tag="pA",
                                   name=f"p_remb{ch}")
                    mms(p_r, [(Vb["lhsT_E1r"], Esb)], cs)
                    p_s = ppA.tile([128, 1024], F32, tag="pA",
                                   name=f"p_s1{ch}")
                    mms(p_s, [(Vb["lhsT_E1s"], Esb)], cs)
                    r2 = pr2.tile([128, 1024], F32, tag="r2",
                                  name=f"r2{ch}")
                    nc.vector.reciprocal_approx_fast(r2[:], p_s[:])
                    nc.vector.tensor_tensor(remb[:, cs], p_r[:], r2[:],
                                            ALU.mult)

                # --- scalar SILU/TANH group ------------------------------
                warm_pe()
                x1 = pha.tile([128, CB], BF16, tag="x1")
                mlp_layer(x1, [(Vb["lhsT_x1"], remb)], AF.Silu,
                          Vf["b_x1"], "p_x1")
                warm_pe()
                a1h0 = pha.tile([128, CB], BF16, tag="a1h0")
                mlp_layer(a1h0, [(Vb["lhsT_d1r0"], remb),
                                 (Vb["lhsT_d1z0"], z4)], AF.Silu,
                          Vf["b_d1h0"], "p_d1a")
                warm_pe()
                a1h1 = pha.tile([128, CB], BF16, tag="a1h1")
                mlp_layer(a1h1, [(Vb["lhsT_d1r1"], remb),
                                 (Vb["lhsT_d1z1"], z4)], AF.Silu,
                          Vf["b_d1h1"], "p_d1b")
                warm_pe()
                a2 = pha.tile([128, CB], BF16, tag="a2")
                mlp_layer(a2, [(Vb["lhsT_d2a0"], a1h0),
                               (Vb["lhsT_d2a1"], a1h1)], AF.Silu,
                          Vf["b_d2"], "p_d2")
                warm_pe()
                th = pha.tile([128, CB], BF16, tag="a1h0", name="th")
                mlp_layer(th, [(Vb["lhsT_gr"], remb),
                               (Vb["lhsT_gz"], z4)], AF.Tanh,
                          Vf["b_g"], "p_g")
                warm_pe()
                cand = pha.tile([128, CB], BF16, tag="a1h1", name="cand")
                mlp_layer(cand, [(Vb["lhsT_cr"], remb),
                                 (Vb["lhsT_cz"], z4)], AF.Tanh,
                          Vf["b_c"], "p_c")
                with tc.high_priority():
                    nc.vector.tensor_scalar(gsil[:], th[0:1, 0:1], 0.0,
                                            1.0e30, ALU.mult, ALU.add)
                    nc.vector.scalar_tensor_tensor(gsil[:], cand[0:1, 0:1],
                                                   0.0, gsil[:], ALU.mult,
                                                   ALU.add)


                # --- alpha (scalar path, silu group) ---------------------
                pa1 = ppt.tile([16, 1], F32, tag="pt", name="pa1")
                nc.tensor.matmul(pa1[:], Vf["lhsT_a1"], Vf["h_col"],
                                 start=True, stop=True)
                # silu via exp (stays in the natural_log_exp set)
                ea = pha.tile([16, 1], F32, tag="ea")
                nc.scalar.activation(ea[:], pa1[:], AF.Exp, scale=-1.0)
                nc.vector.tensor_scalar_add(ea[:], ea[:], 1.0)
                nc.vector.reciprocal(ah[:], ea[:])
                nc.vector.tensor_tensor(ah[:], ah[:], pa1[:], ALU.mult)
                pal = ppt.tile([1, 1], F32, tag="pt", name="pal")
                nc.tensor.matmul(pal[:], Vf["lhsT_a2"], ah[:],
                                 start=True, stop=False)
                nc.tensor.matmul(pal[:], Vf["brow_a2"],
                                 one_col[0:1, 0:1], start=False, stop=True)
                nc.vector.tensor_copy(al_sb[:], pal[:])

                def replicate_col(dst_col, src11, nm):
                    pr = ppt.tile([128, 1], F32, tag="pt", name="rep_" + nm)
                    nc.tensor.matmul(pr[:], ones128[:], src11, start=True,
                                     stop=True)
                    nc.vector.tensor_copy(dst_col[:], pr[:])

                replicate_col(alpha_col, al_sb[:], "alpha")
                nc.vector.tensor_scalar_mul(asc[:], alpha_col[:], INV_SQRT2)
                replicate_col(obs_col, Vf["obs11"], "obs")
                nc.vector.tensor_scalar(rsr[:], rsr[:], 0.15, 2.5,
                                        ALU.max, ALU.min)
                replicate_col(rsrc_col, rsr[:], "rsrc")

                # --- nz = cand + q + th*q, q = z/2 - cand/2 (vector) -----
                ch = pha.tile([128, CB], BF16, tag="Esb", name="ch")
                nc.vector.tensor_scalar_mul(ch[:], cand[:], 0.5)
                q = pha.tile([128, CB], BF16, tag="q")
                nc.vector.tensor_tensor(q[:], z4[:], ch[:], ALU.subtract)
                nz = pha.tile([128, CB], BF16, tag="remb", name="nz")
                nc.vector.tensor_tensor(nz[:], th[:], q[:], ALU.mult)
                nc.vector.tensor_tensor(q[:], q[:], nz[:], ALU.add)
                nc.vector.tensor_tensor(nz[:], cand[:], q[:], ALU.add)

                # --- nlg -> E2 / new_logits (scalar NL group 2) ----------
                E2 = pha.tile([128, CB], BF16, tag="x1", name="E2")
                nlogsb = pha.tile([128, CB], BF16, tag="a1h0",
                                  name="nlogsb")
                for ch in range(NCH):
                    cs = slice(ch * 1024, (ch + 1) * 1024)
                    p_n = ppA.tile([128, 1024], F32, tag="pA",
                                   name=f"p_nlg{ch}")
                    mms(p_n, [(Vb["lhsT_nlgx"], x1),
                              (Vb["lhsT_nlgl"], lg4)], cs)
                    nc.scalar.activation(E2[:, cs], p_n[:], AF.Exp,
                                         bias=Vf["b_nlg"])
                    nc.scalar.activation(nlogsb[:, cs], p_n[:], AF.Identity,
                                         bias=Vf["b_nlg"])

                # scales column: ln(1 + e5) replicated to block rows
                p_rep = ppt.tile([128, 1], F32, tag="pt", name="p_rep")
                nc.tensor.matmul(p_rep[:], Vf["lhsT_rep5"], e5[:],
                                 start=True, stop=True)
                nc.scalar.activation(scl_col[:], p_rep[:], AF.Ln,
                                     bias=one_col[:])
                for b in range(B4):
                    nc.vector.tensor_copy(
                        L_R4[b * 32:b * 32 + K_ACT, 2 * b:2 * b + 1],
                        scl_col[b * 32:b * 32 + K_ACT, 0:1])
                    nc.vector.memset(
                        L_R4[b * 32:b * 32 + 15, 2 * b + 1:2 * b + 2], 1.0)

                # --- d3 + R into one psum tile (rows 0-15 / 32-39) -------
                dpR = pha.tile([40, CB], F32, tag="dpR")
                for ch in range(NCH):
                    cs = slice(ch * 1024, (ch + 1) * 1024)
                    p_dR = ppB.tile([40, 1024], F32, tag="pB",
                                    name=f"p_dR{ch}")
                    mms(p_dR, [(Vb["lhsT_d3"], a2)], cs, rows=slice(0, 16))
                    mms(p_dR, [(L_R4[:], E2)], cs, rows=slice(32, 40))
                    nc.scalar.activation(dpR[:, cs], p_dR[:], AF.Identity,
                                         bias=Vf["b_dR"])

                ln_sup(0, gsil)
                ln_sup(1, gsil)
                ln_sup(2, gsil)
                rsq_sup(0)
                rsq_sup(1)

                # --- transposes -> packed stg6 + state -------------------
                _psk.close()
                _hp = tc.high_priority(offset=100000)
                _hp.__enter__()
                with tc.tile_pool(name="ptr", bufs=3, space="PSUM") as ptr:
                    for t in range(NTC):
                        cs = slice(t * 128, (t + 1) * 128)
                        pT = ptr.tile([128, 40], F32, tag="pT",
                                      name=f"pT{t}")
                        pTb = ptr.tile([128, 256], BF16, tag="pTb",
                                       name=f"pTb{t}")
                        nc.tensor.transpose(pT[:, 0:40], dpR[:, cs],
                                            ident[0:40, 0:40])
                        nc.tensor.transpose(pTb[:, 0:128], nz[:, cs],
                                            identb)
                        nc.tensor.transpose(pTb[:, 128:256], nlogsb[:, cs],
                                            identb)
                        nc.vector.tensor_copy(
                            stg6bv[:, :, 6 * t:6 * t + 4],
                            pT[:, 0:16].rearrange("p (b d) -> p b d", b=B4))
                        nc.vector.tensor_copy(
                            stg6bv[:, :, 6 * t + 4:6 * t + 6],
                            pT[:, 32:40].rearrange("p (b d) -> p b d", b=B4))
                        nc.vector.tensor_copy(
                            statebv[:, :, 50 * t + 2:50 * t + 34],
                            pTb[:, 0:128].rearrange("p (b f) -> p b f",
                                                    b=B4))
                        nc.vector.tensor_copy(
                            statebv[:, :, 50 * t + 34:50 * t + 49],
                            pTb[:, 128:256].rearrange("p (b f) -> p b f",
                                                      b=B4)[:, :, 0:15])

                    # ---- packed scalar chain (all [128, JT]) ------------
                    dp0v = stg6[:, 0:6 * JT:6]
                    dp1v = stg6[:, 1:6 * JT:6]
                    dp2v = stg6[:, 2:6 * JT:6]
                    dp3v = stg6[:, 3:6 * JT:6]
                    Rnv = stg6[:, 4:6 * JT:6]
                    Rdv = stg6[:, 5:6 * JT:6]
                    nhv = hl2[:, 0:2 * JT:2]
                    nlv = hl2[:, 1:2 * JT:2]

                    with tc.tile_pool(name="pk", bufs=12) as pk:
                        def pkt(name):
                            return pk.tile([128, JT], F32, tag="pk",
                                           name=name)

                        def bc(col, n=JT):
                            return col[:, 0:1].to_broadcast([128, n])

                        gtt = nc.gpsimd.tensor_tensor

                        # sig_h/l = softplus(dp2/3)+0.01, h/l paired
                        stg6j = stg6[:, :].rearrange("p (m j) -> p m j",
                                                     j=6)
                        rhrl = Pf[0:128, offs["rh_p"]:offs["rh_p"] + 128] \
                            .rearrange("p (j m) -> p m j", j=2)
                        ehel = Pf[0:128, offs["eh_p"]:offs["eh_p"] + 128] \
                            .rearrange("p (j m) -> p m j", j=2)

                        def pk2(name):
                            t = pk.tile([128, 2 * JT], F32, tag="pk2",
                                        bufs=6, name=name)
                            return t, t[:, :].rearrange(
                                "p (m j) -> p m j", j=2)

                        ex, exj = pk2("ex")
                        nc.scalar.activation(exj, stg6j[:, :, 2:4], AF.Exp)
                        sp, spj = pk2("sp")
                        nc.scalar.activation(sp[:], ex[:], AF.Ln,
                                             bias=one_col[:])
                        m1, m1j = pk2("m1")
                        gtt(m1[:], sp[:], bc(c001, 2 * JT), ALU.add)
                        gtt(m1j, m1j, ehel, ALU.mult)
                        s1, s1j = pk2("s1")
                        gtt(s1j, m1j, rhrl, ALU.add)
                        gtt(s1j, s1j, stg6j[:, :, 0:2], ALU.add)
                        nc.vector.tensor_scalar_max(hl2[:], s1[:], 0.0)

                        # R = clip(R_src * Rn/Rd, .15, 4)
                        rdr = pkt("rdr")
                        nc.vector.reciprocal(rdr[:], Rdv)
                        rr1 = pkt("rr1")
                        gtt(rr1[:], rdr[:], Rnv, ALU.mult)
                        Rv = pkt("Rv")
                        gtt(Rv[:], rr1[:], bc(rsrc_col), ALU.mult)
                        nc.vector.tensor_scalar(Rv[:], Rv[:], 0.15, 4.0,
                                                ALU.max, ALU.min)
                        rcpR = pkt("rcpR")
                        nc.vector.reciprocal(rcpR[:], Rv[:])
                        # zz = (obs - nh)/R ; xw = alpha*zz/sqrt(2)
                        zz = pkt("zz")
                        gtt(zz[:], bc(obs_col), nhv, ALU.subtract)
                        gtt(zz[:], zz[:], rcpR[:], ALU.mult)
                        xw = pkt("xw")
                        gtt(xw[:], zz[:], bc(asc), ALU.mult)
                        # scalar SIG group: just the erf
                        erf_t = pkt("erf_t")
                        nc.scalar.activation(erf_t[:], xw[:], AF.Erf)
                        with tc.high_priority(offset=200000):
                            nc.vector.tensor_scalar(gerf[:],
                                                    erf_t[0:1, 0:1],
                                                    0.0, 1.0e30, ALU.mult,
                                                    ALU.add)

                        # lc = ln(0.5*erf + 0.5) fused into the activation
                        lc = pkt("lc")
                        nc.scalar.activation(lc[:], erf_t[:], AF.Ln,
                                             bias=half_col[:], scale=0.5)
                        lnR = pkt("lnR")
                        nc.scalar.activation(lnR[:], Rv[:], AF.Ln)
                        zz2 = pkt("zz2")
                        gtt(zz2[:], zz[:], zz[:], ALU.mult)
                        gtt(zz2[:], zz2[:], bc(nhalf_col), ALU.mult)
                        l1 = pkt("l1")
                        gtt(l1[:], zz2[:], lc[:], ALU.add)
                        gtt(l1[:], l1[:], lnR[:], ALU.subtract)
                        lw = pkt("lw")
                        gtt(lw[:], l1[:], Vf["lw0_p"], ALU.add)
                        nc.vector.tensor_reduce(lwm[:], lw[:], AX.X,
                                                ALU.max)
                        import concourse.bass_isa as bass_isa
                        nc.gpsimd.partition_all_reduce(
                            lwmax_col[:], lwm[:], 128,
                            bass_isa.ReduceOp.max)
                        dsh = pkt("dsh")
                        nc.vector.tensor_scalar(dsh[:], lw[:],
                                                lwmax_col[:, 0:1], LWCLAMP,
                                                ALU.subtract, ALU.max)
                        nc.scalar.activation(w_p[:], dsh[:], AF.Exp,
                                             scale=2.0)

                    # ---- state: nh/nl cols, ones col, weight fold -------
                    nc.vector.tensor_copy(
                        statemv[:, :, 0:2],
                        hl2[:, :].rearrange("p (m f) -> p m f", m=JT))
                    nc.vector.memset(statemv[:, :, 49:50], 1.0)
                    wb = w_p[:, :].unsqueeze(-1).to_broadcast(
                        [128, JT, 50])
                    nc.gpsimd.tensor_tensor(statemv, statemv, wb, ALU.mult)
                    _hp.__exit__(None, None, None)

                    # remaining big-loop Ln's + fused ops (gated, natural
                    # priority so the chain above always wins ties)
                    for s in range(3, SUP):
                        ln_sup(s, gsil if s < 8 else None)
                    for s in range(2, 8):
                        rsq_sup(s)

            # =================== big loop ================================
            with (
                tc.tile_pool(name="pyp", bufs=1, space="PSUM") as pyp,
                tc.tile_pool(name="pout", bufs=2, space="PSUM") as pout,
                tc.tile_pool(name="outp", bufs=2) as outp,
            ):
                py = pyp.tile([50, R], F32, tag="py")
                for s in range(SUP):
                    if s >= 8:
                        rsq_sup(s)
                    for k in range(G):
                        jt = s * G + k
                        lhsT = state[:, jt * 50:(jt + 1) * 50]
                        for b5 in range(R // 512):
                            rs = slice(k * R + b5 * 512,
                                       k * R + (b5 + 1) * 512)
                            ps = slice(b5 * 512, (b5 + 1) * 512)
                            nc.tensor.matmul(py[:, ps], lhsT,
                                             t_tiles[s][:, rs],
                                             start=(jt == 0),
                                             stop=(jt == JT - 1))

                # ---- output: transpose back, divide by denominator ------
                ysb = outp.tile([50, R], F32, tag="ysb", bufs=1,
                                name="ysb")
                nc.vector.tensor_copy(ysb[:], py[:])
                for ob in range(OB):
                    obs_ = slice(ob * OW, (ob + 1) * OW)
                    po = pout.tile([OW, 50], F32, tag="po", name="po")
                    nc.tensor.transpose(po[:], ysb[:, obs_],
                                        ident[0:50, 0:50])
                    osb = outp.tile([OW, 50], F32, tag="osb", name="osb")
                    nc.scalar.activation(osb[:], po[:], AF.Identity)
                    rden = outp.tile([OW, 1], F32, tag="rden", name="rden")
                    nc.vector.reciprocal(rden[:], osb[:, 49:50])
                    yt = outp.tile([OW, 49], F32, tag="yt", name="yt")
                    nc.scalar.activation(yt[:], osb[:, 0:49], AF.Identity,
                                         scale=rden[:, 0:1])
                    nc.sync.dma_start(d_y[obs_, :], yt[:])
            pfil_cm.__exit__(None, None, None)

        for free in reversed(_keep):
            free()

    nc.compile()
    return nc


# ---------------------------------------------------------------------------
# host-side preparation
# ---------------------------------------------------------------------------

def _f32(x):
    return np.ascontiguousarray(np.asarray(x, dtype=np.float32))


def _bf16(x):
    import ml_dtypes
    return np.ascontiguousarray(np.asarray(x).astype(ml_dtypes.bfloat16))


def prep_inputs(inputs, n_cores):
    g = {k: _f32(v) for k, v in inputs.items()}
    N = g["z"].shape[0]
    JT = N // 128
    CB = N // B4
    R = N // n_cores
    h = g["h_t"]

    def packed(a):
        return np.ascontiguousarray(a.reshape(JT, 128).T)

    W_rt1, W_d1, W_g, W_c = g["W_rt1"], g["W_d1"], g["W_g"], g["W_c"]
    b_rt1 = g["b_rt1"] + W_rt1[:, :64] @ h
    b_d1 = g["b_d1"] + W_d1[:, :64] @ h
    b_g = g["b_g"] + W_g[:, :64] @ h
    b_c = g["b_c"] + W_c[:, :64] @ h

    # block-diagonal builders for the 4-way stacked layout
    def bdiag(blk, rin_pitch, cout_pitch, rtot, ctot):
        out = np.zeros((rtot, ctot), np.float32)
        r, c = blk.shape
        for b in range(B4):
            out[b * rin_pitch:b * rin_pitch + r,
                b * cout_pitch:b * cout_pitch + c] = blk
        return out

    def bias4(vec, pitch=32, rows=128):
        out = np.zeros((rows, 1), np.float32)
        for b in range(B4):
            out[b * pitch:b * pitch + len(vec), 0] = vec
        return out

    # E1: remb_un = E @ embed[:5] ; S1 broadcast to 32 rows per block
    e1r = np.zeros((15, 32), np.float32)
    e1r[:K_ACT, 0:16] = g["embed"][:K_ACT]
    e1s = np.ones((15, 32), np.float32)
    lhsT_E1r = bdiag(e1r, 15, 32, 60, 128)
    lhsT_E1s = bdiag(e1s, 15, 32, 60, 128)

    # x1 = silu(W_rt1[:, 64:80] . remb + b)
    x1blk = np.zeros((32, 32), np.float32)
    x1blk[0:16, :] = W_rt1[:, 64:80].T
    lhsT_x1 = bdiag(x1blk, 32, 32, 128, 128)

    # nlg: new_logits = 0.3*W_rt2.x1 (first 5) + {0.7,1.0}*logits
    nlgx = np.zeros((32, 32), np.float32)
    nlgx[:, :K_ACT] = 0.3 * g["W_rt2"].T[:, :K_ACT]
    lhsT_nlgx = bdiag(nlgx, 32, 32, 128, 128)
    nlgl = np.zeros((15, 32), np.float32)
    for j in range(15):
        nlgl[j, j] = 0.7 if j < K_ACT else 1.0
    lhsT_nlgl = bdiag(nlgl, 15, 32, 60, 128)
    b_nlg = np.zeros(32, np.float32)
    b_nlg[:K_ACT] = 0.3 * g["b_rt2"][:K_ACT]

    # d1 halves: remb part (rows 0-15) and z part
    def dh(W, lo, hi, src):   # src: 64..80 remb / 80..112 z
        blk = np.zeros((32 if src == "r" else 32, 32), np.float32)
        if src == "r":
            blk = np.zeros((32, 32), np.float32)
            blk[0:16, :] = W[lo:hi, 64:80].T
        else:
            blk = W[lo:hi, 80:112].T
        return bdiag(blk, 32, 32, 128, 128)

    lhsT_d1r0 = dh(W_d1, 0, 32, "r")
    lhsT_d1z0 = dh(2.0 * W_d1, 0, 32, "z")
    lhsT_d1r1 = dh(W_d1, 32, 64, "r")
    lhsT_d1z1 = dh(2.0 * W_d1, 32, 64, "z")
    lhsT_d2a0 = bdiag(g["W_d2"][:, 0:32].T, 32, 32, 128, 128)
    lhsT_d2a1 = bdiag(g["W_d2"][:, 32:64].T, 32, 32, 128, 128)
    lhsT_d3 = bdiag(g["W_d3"].T, 32, 4, 128, 16)
    lhsT_gr = dh(0.5 * W_g, 0, 32, "r")
    lhsT_gz = dh(W_g, 0, 32, "z")
    lhsT_cr = dh(W_c, 0, 32, "r")
    lhsT_cz = dh(2.0 * W_c, 0, 32, "z")

    b_dR = np.zeros((40, 1), np.float32)
    for b in range(B4):
        b_dR[b * 4:b * 4 + 4, 0] = g["b_d3"]

    lhsT_rep5 = np.zeros((5, 128), np.float32)
    for b in range(B4):
        for j in range(K_ACT):
            lhsT_rep5[j, b * 32 + j] = 1.0

    lhsT_a1 = np.concatenate([g["W_a1"].T, g["b_a1"][None, :]], 0)
    h_colv = np.concatenate([h, np.ones(1, np.float32)])[:, None]

    piecesf = {
        "ident": np.eye(128, dtype=np.float32),
        "lhsT_a1": _f32(lhsT_a1), "lhsT_a2": _f32(g["W_a2"].T),
        "brow_a2": _f32(g["b_a2"][None, :]), "h_col": _f32(h_colv),
        "lhsT_rep5": lhsT_rep5,
        "log_obs5": _f32(g["log_obs_scale"][:K_ACT][:, None]),
        "logR0": _f32(g["log_R"][0].reshape(1, 1)),
        "obs11": _f32(np.asarray(g["obs_remaining"]).reshape(1, 1)),
        "b_x1": bias4(b_rt1), "b_d1h0": bias4(b_d1[0:32]),
        "b_d1h1": bias4(b_d1[32:64]), "b_d2": bias4(g["b_d2"]),
        "b_dR": b_dR, "b_nlg": bias4(b_nlg),
        "b_g": bias4(0.5 * b_g), "b_c": bias4(b_c),
        "rh_p": packed(g["remaining_high"]),
        "rlow_p": packed(g["remaining_low"]),
        "eh_p": packed(g["eps_high"]),
        "el_p": packed(g["eps_low"]),
        "lw0_p": packed(g["log_weights"]) + np.float32(C_LL),
    }
    piecesb = {
        "identb": np.eye(128, dtype=np.float32),
        "lhsT_E1r": lhsT_E1r, "lhsT_E1s": lhsT_E1s, "lhsT_x1": lhsT_x1,
        "lhsT_nlgx": lhsT_nlgx, "lhsT_nlgl": lhsT_nlgl,
        "lhsT_d1r0": lhsT_d1r0, "lhsT_d1z0": lhsT_d1z0,
        "lhsT_d1r1": lhsT_d1r1, "lhsT_d1z1": lhsT_d1z1,
        "lhsT_d2a0": lhsT_d2a0, "lhsT_d2a1": lhsT_d2a1,
        "lhsT_d3": lhsT_d3,
        "lhsT_gr": lhsT_gr, "lhsT_gz": lhsT_gz,
        "lhsT_cr": lhsT_cr, "lhsT_cz": lhsT_cz,
    }

    import ml_dtypes

    def pack_blob(spec, pieces, dt):
        CP = sum(m for _, _, m in spec)
        blob = np.zeros((128, CP), dt)
        off = 0
        for nm, k, m in spec:
            arr = pieces[nm]
            assert arr.shape == (k, m), (nm, arr.shape, (k, m))
            blob[0:k, off:off + m] = arr.astype(dt)
            off += m
        return blob

    pf = pack_blob(_param_spec_f32(), piecesf, np.float32)
    pb = pack_blob(_param_spec_bf16(), piecesb, ml_dtypes.bfloat16)

    # 4-way stacked activations (bf16)
    z4 = np.ascontiguousarray(
        (0.5 * g["z"]).reshape(B4, CB, 32).transpose(0, 2, 1)
        .reshape(128, CB))
    lg4 = np.ascontiguousarray(
        g["regime_logits"].reshape(B4, CB, 15).transpose(0, 2, 1)
        .reshape(60, CB))

    common = dict(
        z4=_bf16(z4),
        lg4=_bf16(lg4),
        pf=pf,
        pb=np.ascontiguousarray(pb),
    )

    u = g["u_gumbel"]
    in_maps = []
    for c in range(n_cores):
        m = dict(common)
        m["uT"] = np.ascontiguousarray(u[c * R:(c + 1) * R, :].T)
        in_maps.append(m)
    return in_maps


_PROG_CACHE = {}
TRACE = False           # set True (e.g. from test.py) to profile on HW
LAST_EXEC_NS = None


def kernel(**inputs):
    global LAST_EXEC_NS
    n_cores = 8
    N = int(np.asarray(inputs["z"]).shape[0])
    R = N // n_cores
    key = (N, R)
    if key not in _PROG_CACHE:
        _PROG_CACHE[key] = build_program(N, R)
    nc = _PROG_CACHE[key]
    in_maps = prep_inputs(inputs, n_cores)
    res = run_bass_kernel_spmd(nc, in_maps, list(range(n_cores)),
                               trace=TRACE)
    LAST_EXEC_NS = res.exec_time_ns
    outs = [res.results[c]["y"] for c in range(n_cores)]
    return np.concatenate(outs, axis=0).astype(np.float32)
